# revision 3
# baseline (speedup 1.0000x reference)
"""Int8-dequant linear layer (out = input @ (qweight*scale).T + bias) on 8 trn2 cores.

Token-parallel: each core computes 512 tokens against the full weight matrix.
v2: activations ship pre-transposed fp16 from the host (no on-device PE
transposes, no fp32 x traffic) and weights ship as fp16 in DRAM (plain
line-rate DMA instead of the slow int8->fp16 cast path). The kernel is then
purely PE-bound: 1024 bf16-rate matmuls of N=512 per core with a ~5us DMA
lead-in. Scale and bias are applied in fp32 (ScalarE mul + VectorE add) after
fp32 PSUM accumulation; the only precision loss vs the fp32 reference is the
fp16 rounding of the activations (~2e-4 relative).
"""

import numpy as np

B, S, IN_F, OUT_F = 8, 512, 4096, 4096
N_CORES = 8
TOK = B * S                # 4096 tokens total
TOK_C = TOK // N_CORES     # 512 tokens per core
P = 128                    # partitions
KT = IN_F // P             # 32 k-tiles
NT = 512                   # out-feature chunk (one fp32 PSUM bank)
OF_CHUNKS = OUT_F // NT    # 8
TT = TOK_C // P            # 4 token tiles per core


def _make_tile_context_cls():
    import bass_rust
    import concourse.mybir as mybir
    from concourse.tile import TileContext, ScopedClock

    class _TC(TileContext):
        # The walrus build in this image rejects more than one semaphore wait
        # per instruction. Split extra waits onto nofuse NOPs committed just
        # before the instruction on the same engine (identical queue
        # semantics: the sequencer blocks on the NOP's wait first).
        def _commit_instruction(self, inst, lazy_reg_writes: bool = True):
            si = getattr(inst, "sync_info", None)
            if (
                si is not None
                and len(si.on_wait) > 1
                and inst.engine != mybir.EngineType.Unassigned
            ):
                waits = list(si.on_wait)
                for i, w in enumerate(waits[:-1]):
                    nop = mybir.InstNoOp(
                        name=f"{inst.name}-ws{i}",
                        sync_info=mybir.SyncInfo(on_wait=[w], on_update=[]),
                        bass_nofuse=True,
                        engine=inst.engine,
                    )
                    self._add_instruction(nop)
                inst.sync_info = mybir.SyncInfo(
                    on_wait=[waits[-1]], on_update=list(si.on_update)
                )
            return super()._commit_instruction(inst, lazy_reg_writes)

        # Same walrus limitation: it can't encode syncs on the exit Drain, so
        # land the end-of-kernel clock waits on single-wait NOPs and use the
        # sequencer-level (EVSEM-only) barrier instead of the drain butterfly.
        def _drain_and_barrier(self, tick_clock, wait_clock):
            nc = self.nc
            carrier = nc.sync.nop(nofuse=True)
            wait_clock.add_sem_waits(
                carrier.ins, ScopedClock({None: tick_clock.global_clock})
            )
            waits = list(carrier.ins.sync_info.on_wait)
            if len(waits) > 1:
                carrier.ins.sync_info = bass_rust.SyncInfo(
                    on_wait=[waits[0]], on_update=[]
                )
                for w in waits[1:]:
                    extra = nc.sync.nop(nofuse=True)
                    extra.ins.sync_info = bass_rust.SyncInfo(
                        on_wait=[w], on_update=[]
                    )
            nc.sync.drain()
            nc.all_engine_barrier(sem_only=True)
            assert self.sems is not None
            popped = nc._tile_sem_poison_stack.pop()
            assert popped is self._sem_poison
            nc.clear_and_free_semaphores(list(self.sems.allocated().values()))
            nc.all_engine_barrier(sem_only=True)

    return _TC


def build_nc():
    """Build the per-core Bass program (SPMD: same program, different x shard)."""
    import concourse.bass as bass
    import concourse.mybir as mybir

    f16 = mybir.dt.float16
    f32 = mybir.dt.float32

    nc = bass.Bass("TRN2", target_bir_lowering=False, debug=False)
    # xt[p, j, t] = fp16(x[t, j*128+p]) : activations pre-transposed on host
    xt = nc.dram_tensor("xt", [P, KT, TOK_C], f16, kind="ExternalInput").ap()
    # w_packed[of, p, j, n] = fp16(qweight[of*NT + n, j*P + p])
    wt = nc.dram_tensor(
        "wt", [OF_CHUNKS, P, KT, NT], f16, kind="ExternalInput"
    ).ap()
    # bias comes pre-broadcast to 128 partitions from the host: a plain
    # contiguous 2MB DMA is much faster than a [1,N]->[128,N] broadcast DMA.
    bias = nc.dram_tensor("bias", [P, OUT_F], f32, kind="ExternalInput").ap()
    scale = nc.dram_tensor("scale", [1, 1], f32, kind="ExternalInput").ap()
    out = nc.dram_tensor("out", [TOK_C, OUT_F], f32, kind="ExternalOutput").ap()

    TC = _make_tile_context_cls()
    with TC(nc) as tc:
        with (
            tc.tile_pool(name="persist", bufs=1) as persist,
            tc.tile_pool(name="wpool", bufs=3) as wpool,
            tc.tile_pool(name="opool", bufs=6) as opool,
            tc.tile_pool(name="pacc", bufs=6, space="PSUM") as pacc_pool,
            tc.tile_pool(name="pwarm", bufs=1, space="PSUM") as pwarm_pool,
        ):
            # activations first on the HWDGE queue (4 x 1MB, j-major) so the
            # first matmuls can start after ~1MB; scale/bias follow on the
            # same queue and are only needed by the first epilogue (~30us in)
            xt_sb = persist.tile([P, KT, TOK_C], f16)
            for q in range(8):
                nc.sync.dma_start(
                    out=xt_sb[:, q * (KT // 8):(q + 1) * (KT // 8), :],
                    in_=xt[:, q * (KT // 8):(q + 1) * (KT // 8), :],
                )
            scale_sb = persist.tile([P, 1], f32)
            nc.sync.dma_start(out=scale_sb, in_=scale.to_broadcast((P, 1)))
            bias_sb = persist.tile([P, OUT_F], f32)
            nc.sync.dma_start(out=bias_sb, in_=bias)

            # HAM warm-up: dummy matmuls on the first resident xt block keep
            # the PE busy during the DMA lead-in so the SHORT window flips
            # the clock gate to 8/8 before the real matmuls begin.
            warm = pwarm_pool.tile([P, P], f32)
            for r in range(24):
                nc.tensor.matmul(
                    warm,
                    lhsT=xt_sb[:, 0, 0:P],
                    rhs=xt_sb[:, 0, 0:P],
                    start=(r == 0),
                    stop=(r == 23),
                )

            # weights stream on the gpsimd (SWDGE) queue, overlapping the
            # activation loads; chunk 0 is split so matmul j can start once
            # its k-block is resident
            for of in range(OF_CHUNKS):
                wc = wpool.tile([P, KT, NT], f16)
                if of == 0:
                    for q in range(8):
                        nc.gpsimd.dma_start(
                            out=wc[:, q * (KT // 8):(q + 1) * (KT // 8), :],
                            in_=wt[of, :, q * (KT // 8):(q + 1) * (KT // 8), :],
                        )
                else:
                    nc.gpsimd.dma_start(out=wc, in_=wt[of])
                for t in range(TT):
                    acc = pacc_pool.tile([P, NT], f32)
                    for j in range(KT):
                        nc.tensor.matmul(
                            acc,
                            lhsT=xt_sb[:, j, t * P:(t + 1) * P],
                            rhs=wc[:, j, :],
                            start=(j == 0),
                            stop=(j == KT - 1),
                        )
                    osb = opool.tile([P, NT], f32)
                    nc.scalar.mul(osb, acc, scale_sb[:, :])
                    nc.vector.tensor_add(osb, osb, bias_sb[:, of * NT:(of + 1) * NT])
                    nc.sync.dma_start(
                        out=out[t * P:(t + 1) * P, of * NT:(of + 1) * NT], in_=osb
                    )
    return nc


def prep_inputs(input, qweight, weight_scale, bias_param):
    """Host-side shard/repack. Returns per-core in_maps."""
    X = np.asarray(input, dtype=np.float32).reshape(TOK, IN_F)
    # w fp16 (exact for int8 range), packed [of, p, j, n]
    wp = np.ascontiguousarray(
        np.asarray(qweight, dtype=np.float16)
        .reshape(OF_CHUNKS, NT, KT, P)
        .transpose(0, 3, 2, 1)
    )
    bias2 = np.ascontiguousarray(
        np.broadcast_to(
            np.asarray(bias_param, dtype=np.float32).reshape(1, OUT_F), (P, OUT_F)
        )
    )
    scale2 = np.ascontiguousarray(
        np.asarray(weight_scale, dtype=np.float32).reshape(1, 1)
    )
    in_maps = []
    for c in range(N_CORES):
        xc = X[c * TOK_C:(c + 1) * TOK_C]  # [TOK_C, IN_F]
        # xt[p, j, t] = x[t, j*128+p]
        xtc = np.ascontiguousarray(
            xc.reshape(TOK_C, KT, P).transpose(2, 1, 0).astype(np.float16)
        )
        in_maps.append({"xt": xtc, "wt": wp, "bias": bias2, "scale": scale2})
    return in_maps


def assemble_output(results):
    out = np.concatenate([results[c]["out"] for c in range(N_CORES)], axis=0)
    return np.ascontiguousarray(out.reshape(B, S, OUT_F).astype(np.float32))


def kernel(input, qweight, weight_scale, bias_param):
    from concourse.bass_utils import run_bass_kernel_spmd

    in_maps = prep_inputs(input, qweight, weight_scale, bias_param)
    nc = build_nc()
    res = run_bass_kernel_spmd(nc, in_maps, core_ids=list(range(N_CORES)))
    return assemble_output(res.results)


# revision 4
# speedup vs baseline: 1.5320x; 1.5320x over previous
"""Int8-dequant linear layer (out = input @ (qweight*scale).T + bias) on 8 trn2 cores.

Token-parallel: each core computes 512 tokens against the full weight matrix.
v2: activations ship pre-transposed fp16 from the host (no on-device PE
transposes, no fp32 x traffic) and weights ship as fp16 in DRAM (plain
line-rate DMA instead of the slow int8->fp16 cast path). The kernel is then
purely PE-bound: 1024 bf16-rate matmuls of N=512 per core with a ~5us DMA
lead-in. Scale and bias are applied in fp32 (ScalarE mul + VectorE add) after
fp32 PSUM accumulation; the only precision loss vs the fp32 reference is the
fp16 rounding of the activations (~2e-4 relative).
"""

import numpy as np

B, S, IN_F, OUT_F = 8, 512, 4096, 4096
N_CORES = 8
TOK = B * S                # 4096 tokens total
TOK_C = TOK // N_CORES     # 512 tokens per core
P = 128                    # partitions
KT = IN_F // P             # 32 k-tiles
NT = 512                   # out-feature chunk (one fp32 PSUM bank)
OF_CHUNKS = OUT_F // NT    # 8
TT = TOK_C // P            # 4 token tiles per core


def _make_tile_context_cls():
    import bass_rust
    import concourse.mybir as mybir
    from concourse.tile import TileContext, ScopedClock

    class _TC(TileContext):
        # The walrus build in this image rejects more than one semaphore wait
        # per instruction. Split extra waits onto nofuse NOPs committed just
        # before the instruction on the same engine (identical queue
        # semantics: the sequencer blocks on the NOP's wait first).
        def _commit_instruction(self, inst, lazy_reg_writes: bool = True):
            si = getattr(inst, "sync_info", None)
            if (
                si is not None
                and len(si.on_wait) > 1
                and inst.engine != mybir.EngineType.Unassigned
            ):
                waits = list(si.on_wait)
                for i, w in enumerate(waits[:-1]):
                    nop = mybir.InstNoOp(
                        name=f"{inst.name}-ws{i}",
                        sync_info=mybir.SyncInfo(on_wait=[w], on_update=[]),
                        bass_nofuse=True,
                        engine=inst.engine,
                    )
                    self._add_instruction(nop)
                inst.sync_info = mybir.SyncInfo(
                    on_wait=[waits[-1]], on_update=list(si.on_update)
                )
            return super()._commit_instruction(inst, lazy_reg_writes)

        # Same walrus limitation: it can't encode syncs on the exit Drain, so
        # land the end-of-kernel clock waits on single-wait NOPs and use the
        # sequencer-level (EVSEM-only) barrier instead of the drain butterfly.
        def _drain_and_barrier(self, tick_clock, wait_clock):
            nc = self.nc
            carrier = nc.sync.nop(nofuse=True)
            wait_clock.add_sem_waits(
                carrier.ins, ScopedClock({None: tick_clock.global_clock})
            )
            waits = list(carrier.ins.sync_info.on_wait)
            if len(waits) > 1:
                carrier.ins.sync_info = bass_rust.SyncInfo(
                    on_wait=[waits[0]], on_update=[]
                )
                for w in waits[1:]:
                    extra = nc.sync.nop(nofuse=True)
                    extra.ins.sync_info = bass_rust.SyncInfo(
                        on_wait=[w], on_update=[]
                    )
            nc.sync.drain()
            nc.all_engine_barrier(sem_only=True)
            assert self.sems is not None
            popped = nc._tile_sem_poison_stack.pop()
            assert popped is self._sem_poison
            nc.clear_and_free_semaphores(list(self.sems.allocated().values()))
            nc.all_engine_barrier(sem_only=True)

    return _TC


def build_nc():
    """Build the per-core Bass program (SPMD: same program, different x shard)."""
    import concourse.bass as bass
    import concourse.mybir as mybir

    f16 = mybir.dt.float16
    f32 = mybir.dt.float32

    nc = bass.Bass("TRN2", target_bir_lowering=False, debug=False)
    # xt[p, j, t] = fp16(x[t, j*128+p]) : activations pre-transposed on host
    xt = nc.dram_tensor("xt", [P, KT, TOK_C], f16, kind="ExternalInput").ap()
    # w_packed[of, p, j, n] = fp16(qweight[of*NT + n, j*P + p])
    wt = nc.dram_tensor(
        "wt", [OF_CHUNKS, P, KT, NT], f16, kind="ExternalInput"
    ).ap()
    # bias comes pre-broadcast to 128 partitions from the host: a plain
    # contiguous 2MB DMA is much faster than a [1,N]->[128,N] broadcast DMA.
    bias = nc.dram_tensor("bias", [P, OUT_F], f32, kind="ExternalInput").ap()
    scale = nc.dram_tensor("scale", [1, 1], f32, kind="ExternalInput").ap()
    out = nc.dram_tensor("out", [TOK_C, OUT_F], f32, kind="ExternalOutput").ap()

    TC = _make_tile_context_cls()
    with TC(nc) as tc:
        with (
            tc.tile_pool(name="persist", bufs=1) as persist,
            tc.tile_pool(name="wpool", bufs=3) as wpool,
            tc.tile_pool(name="opool", bufs=6) as opool,
            tc.tile_pool(name="pacc", bufs=6, space="PSUM") as pacc_pool,
            tc.tile_pool(name="pwarm", bufs=1, space="PSUM") as pwarm_pool,
        ):
            # activations first on the HWDGE queue (4 x 1MB, j-major) so the
            # first matmuls can start after ~1MB; scale/bias follow on the
            # same queue and are only needed by the first epilogue (~30us in)
            xt_sb = persist.tile([P, KT, TOK_C], f16)
            for q in range(8):
                nc.sync.dma_start(
                    out=xt_sb[:, q * (KT // 8):(q + 1) * (KT // 8), :],
                    in_=xt[:, q * (KT // 8):(q + 1) * (KT // 8), :],
                )
            scale_sb = persist.tile([P, 1], f32)
            nc.sync.dma_start(out=scale_sb, in_=scale.to_broadcast((P, 1)))
            bias_sb = persist.tile([P, OUT_F], f32)
            nc.sync.dma_start(out=bias_sb, in_=bias)

            # HAM warm-up: dummy matmuls on a zeroed tile keep the PE busy
            # from t~0 during the DMA lead-in so the SHORT window flips the
            # clock gate to 8/8 before the real matmuls begin. The source is
            # memzero'd SBUF (no DMA dependency), so warm-up starts
            # immediately rather than after the first activation chunk lands.
            warm_src = persist.tile([P, P], f16)
            nc.vector.memzero(warm_src)
            warm = pwarm_pool.tile([P, P], f32)
            for r in range(32):
                nc.tensor.matmul(
                    warm,
                    lhsT=warm_src,
                    rhs=warm_src,
                    start=(r == 0),
                    stop=(r == 31),
                )

            # weights stream on the gpsimd (SWDGE) queue, overlapping the
            # activation loads; chunk 0 is split so matmul j can start once
            # its k-block is resident
            for of in range(OF_CHUNKS):
                wc = wpool.tile([P, KT, NT], f16)
                if of == 0:
                    for q in range(8):
                        nc.gpsimd.dma_start(
                            out=wc[:, q * (KT // 8):(q + 1) * (KT // 8), :],
                            in_=wt[of, :, q * (KT // 8):(q + 1) * (KT // 8), :],
                        )
                else:
                    # halves: finer completion gating so the chunk's first
                    # matmuls are released before the whole 4MB lands
                    for q in range(2):
                        nc.gpsimd.dma_start(
                            out=wc[:, q * (KT // 2):(q + 1) * (KT // 2), :],
                            in_=wt[of, :, q * (KT // 2):(q + 1) * (KT // 2), :],
                        )
                for t in range(TT):
                    acc = pacc_pool.tile([P, NT], f32)
                    for j in range(KT):
                        nc.tensor.matmul(
                            acc,
                            lhsT=xt_sb[:, j, t * P:(t + 1) * P],
                            rhs=wc[:, j, :],
                            start=(j == 0),
                            stop=(j == KT - 1),
                        )
                    osb = opool.tile([P, NT], f32)
                    nc.scalar.mul(osb, acc, scale_sb[:, :])
                    nc.vector.tensor_add(osb, osb, bias_sb[:, of * NT:(of + 1) * NT])
                    nc.sync.dma_start(
                        out=out[t * P:(t + 1) * P, of * NT:(of + 1) * NT], in_=osb
                    )
    return nc


def prep_inputs(input, qweight, weight_scale, bias_param):
    """Host-side shard/repack. Returns per-core in_maps."""
    X = np.asarray(input, dtype=np.float32).reshape(TOK, IN_F)
    # w fp16 (exact for int8 range), packed [of, p, j, n]
    wp = np.ascontiguousarray(
        np.asarray(qweight, dtype=np.float16)
        .reshape(OF_CHUNKS, NT, KT, P)
        .transpose(0, 3, 2, 1)
    )
    bias2 = np.ascontiguousarray(
        np.broadcast_to(
            np.asarray(bias_param, dtype=np.float32).reshape(1, OUT_F), (P, OUT_F)
        )
    )
    scale2 = np.ascontiguousarray(
        np.asarray(weight_scale, dtype=np.float32).reshape(1, 1)
    )
    in_maps = []
    for c in range(N_CORES):
        xc = X[c * TOK_C:(c + 1) * TOK_C]  # [TOK_C, IN_F]
        # xt[p, j, t] = x[t, j*128+p]
        xtc = np.ascontiguousarray(
            xc.reshape(TOK_C, KT, P).transpose(2, 1, 0).astype(np.float16)
        )
        in_maps.append({"xt": xtc, "wt": wp, "bias": bias2, "scale": scale2})
    return in_maps


def assemble_output(results):
    out = np.concatenate([results[c]["out"] for c in range(N_CORES)], axis=0)
    return np.ascontiguousarray(out.reshape(B, S, OUT_F).astype(np.float32))


def kernel(input, qweight, weight_scale, bias_param):
    from concourse.bass_utils import run_bass_kernel_spmd

    in_maps = prep_inputs(input, qweight, weight_scale, bias_param)
    nc = build_nc()
    res = run_bass_kernel_spmd(nc, in_maps, core_ids=list(range(N_CORES)))
    return assemble_output(res.results)


# revision 5
# speedup vs baseline: 3.1947x; 2.0853x over previous
"""Int8-dequant linear layer (out = input @ (qweight*scale).T + bias) on 8 trn2 cores.

Token-parallel: each core computes 512 tokens against the full weight matrix.
v2: activations ship pre-transposed fp16 from the host (no on-device PE
transposes, no fp32 x traffic) and weights ship as fp16 in DRAM (plain
line-rate DMA instead of the slow int8->fp16 cast path). The kernel is then
purely PE-bound: 1024 bf16-rate matmuls of N=512 per core with a ~5us DMA
lead-in. Scale and bias are applied in fp32 (ScalarE mul + VectorE add) after
fp32 PSUM accumulation; the only precision loss vs the fp32 reference is the
fp16 rounding of the activations (~2e-4 relative).
"""

import numpy as np

B, S, IN_F, OUT_F = 8, 512, 4096, 4096
N_CORES = 8
TOK = B * S                # 4096 tokens total
TOK_C = TOK // N_CORES     # 512 tokens per core
P = 128                    # partitions
KT = IN_F // P             # 32 k-tiles
NT = 512                   # out-feature chunk (one fp32 PSUM bank)
OF_CHUNKS = OUT_F // NT    # 8
TT = TOK_C // P            # 4 token tiles per core


def _make_tile_context_cls():
    import bass_rust
    import concourse.mybir as mybir
    from concourse.tile import TileContext, ScopedClock

    class _TC(TileContext):
        # The walrus build in this image rejects more than one semaphore wait
        # per instruction. Split extra waits onto nofuse NOPs committed just
        # before the instruction on the same engine (identical queue
        # semantics: the sequencer blocks on the NOP's wait first).
        def _commit_instruction(self, inst, lazy_reg_writes: bool = True):
            si = getattr(inst, "sync_info", None)
            if (
                si is not None
                and len(si.on_wait) > 1
                and inst.engine != mybir.EngineType.Unassigned
            ):
                waits = list(si.on_wait)
                for i, w in enumerate(waits[:-1]):
                    nop = mybir.InstNoOp(
                        name=f"{inst.name}-ws{i}",
                        sync_info=mybir.SyncInfo(on_wait=[w], on_update=[]),
                        bass_nofuse=True,
                        engine=inst.engine,
                    )
                    self._add_instruction(nop)
                inst.sync_info = mybir.SyncInfo(
                    on_wait=[waits[-1]], on_update=list(si.on_update)
                )
            return super()._commit_instruction(inst, lazy_reg_writes)

        # Same walrus limitation: it can't encode syncs on the exit Drain, so
        # land the end-of-kernel clock waits on single-wait NOPs and use the
        # sequencer-level (EVSEM-only) barrier instead of the drain butterfly.
        def _drain_and_barrier(self, tick_clock, wait_clock):
            nc = self.nc
            carrier = nc.sync.nop(nofuse=True)
            wait_clock.add_sem_waits(
                carrier.ins, ScopedClock({None: tick_clock.global_clock})
            )
            waits = list(carrier.ins.sync_info.on_wait)
            if len(waits) > 1:
                carrier.ins.sync_info = bass_rust.SyncInfo(
                    on_wait=[waits[0]], on_update=[]
                )
                for w in waits[1:]:
                    extra = nc.sync.nop(nofuse=True)
                    extra.ins.sync_info = bass_rust.SyncInfo(
                        on_wait=[w], on_update=[]
                    )
            nc.sync.drain()
            nc.all_engine_barrier(sem_only=True)
            assert self.sems is not None
            popped = nc._tile_sem_poison_stack.pop()
            assert popped is self._sem_poison
            nc.clear_and_free_semaphores(list(self.sems.allocated().values()))
            nc.all_engine_barrier(sem_only=True)

    return _TC


def build_nc():
    """Build the per-core Bass program (SPMD: same program, different x shard)."""
    import concourse.bass as bass
    import concourse.mybir as mybir

    f16 = mybir.dt.float16
    f32 = mybir.dt.float32

    nc = bass.Bass("TRN2", target_bir_lowering=False, debug=False)
    # xt[p, j, t] = fp16(x[t, j*128+p]) : activations pre-transposed on host
    xt = nc.dram_tensor("xt", [P, KT, TOK_C], f16, kind="ExternalInput").ap()
    # w_packed[of, p, j, n] = fp16(qweight[of*NT + n, j*P + p])
    wt = nc.dram_tensor(
        "wt", [OF_CHUNKS, P, KT, NT], f16, kind="ExternalInput"
    ).ap()
    # bias comes pre-broadcast to 128 partitions from the host: a plain
    # contiguous 2MB DMA is much faster than a [1,N]->[128,N] broadcast DMA.
    bias = nc.dram_tensor("bias", [P, OUT_F], f32, kind="ExternalInput").ap()
    scale = nc.dram_tensor("scale", [1, 1], f32, kind="ExternalInput").ap()
    out = nc.dram_tensor("out", [TOK_C, OUT_F], f32, kind="ExternalOutput").ap()

    TC = _make_tile_context_cls()
    with TC(nc) as tc:
        with (
            tc.tile_pool(name="persist", bufs=1) as persist,
            tc.tile_pool(name="wpool", bufs=4) as wpool,
            tc.tile_pool(name="opool", bufs=6) as opool,
            tc.tile_pool(name="pacc", bufs=6, space="PSUM") as pacc_pool,
            tc.tile_pool(name="pwarm", bufs=1, space="PSUM") as pwarm_pool,
        ):
            # activations first on the HWDGE queue (4 x 1MB, j-major) so the
            # first matmuls can start after ~1MB; scale/bias follow on the
            # same queue and are only needed by the first epilogue (~30us in)
            xt_sb = persist.tile([P, KT, TOK_C], f16)
            for q in range(8):
                nc.sync.dma_start(
                    out=xt_sb[:, q * (KT // 8):(q + 1) * (KT // 8), :],
                    in_=xt[:, q * (KT // 8):(q + 1) * (KT // 8), :],
                )
            scale_sb = persist.tile([P, 1], f32)
            nc.sync.dma_start(out=scale_sb, in_=scale.to_broadcast((P, 1)))
            bias_sb = persist.tile([P, OUT_F], f32)
            nc.sync.dma_start(out=bias_sb, in_=bias)

            # HAM warm-up: dummy matmuls on a zeroed tile keep the PE busy
            # from t~0 during the DMA lead-in so the SHORT window flips the
            # clock gate to 8/8 before the real matmuls begin. The source is
            # memzero'd SBUF (no DMA dependency), so warm-up starts
            # immediately rather than after the first activation chunk lands.
            warm_src = persist.tile([P, P], f16)
            nc.vector.memzero(warm_src)
            warm = pwarm_pool.tile([P, P], f32)
            for r in range(32):
                nc.tensor.matmul(
                    warm,
                    lhsT=warm_src,
                    rhs=warm_src,
                    start=(r == 0),
                    stop=(r == 31),
                )

            # weights stream on the gpsimd (SWDGE) queue, overlapping the
            # activation loads; chunk 0 is split so matmul j can start once
            # its k-block is resident
            for of in range(OF_CHUNKS):
                # weights stream on TWO independent DMA paths — gpsimd
                # (SWDGE) and scalar (the second HWDGE ring on trn2) — in
                # alternating j-ranges, doubling weight-stream bandwidth and
                # halving each chunk's arrival time; sequential j-ranges keep
                # delivery in consumption order for fine-grained gating.
                wc = wpool.tile([P, KT, NT], f16)
                nsplit = 8 if of == 0 else 2
                step = KT // nsplit
                for q in range(nsplit):
                    eng = nc.gpsimd if q % 2 == 0 else nc.scalar
                    eng.dma_start(
                        out=wc[:, q * step:(q + 1) * step, :],
                        in_=wt[of, :, q * step:(q + 1) * step, :],
                    )
                for t in range(TT):
                    acc = pacc_pool.tile([P, NT], f32)
                    for j in range(KT):
                        nc.tensor.matmul(
                            acc,
                            lhsT=xt_sb[:, j, t * P:(t + 1) * P],
                            rhs=wc[:, j, :],
                            start=(j == 0),
                            stop=(j == KT - 1),
                        )
                    osb = opool.tile([P, NT], f32)
                    nc.scalar.mul(osb, acc, scale_sb[:, :])
                    nc.vector.tensor_add(osb, osb, bias_sb[:, of * NT:(of + 1) * NT])
                    nc.sync.dma_start(
                        out=out[t * P:(t + 1) * P, of * NT:(of + 1) * NT], in_=osb
                    )
    return nc


def prep_inputs(input, qweight, weight_scale, bias_param):
    """Host-side shard/repack. Returns per-core in_maps."""
    X = np.asarray(input, dtype=np.float32).reshape(TOK, IN_F)
    # w fp16 (exact for int8 range), packed [of, p, j, n]
    wp = np.ascontiguousarray(
        np.asarray(qweight, dtype=np.float16)
        .reshape(OF_CHUNKS, NT, KT, P)
        .transpose(0, 3, 2, 1)
    )
    bias2 = np.ascontiguousarray(
        np.broadcast_to(
            np.asarray(bias_param, dtype=np.float32).reshape(1, OUT_F), (P, OUT_F)
        )
    )
    scale2 = np.ascontiguousarray(
        np.asarray(weight_scale, dtype=np.float32).reshape(1, 1)
    )
    in_maps = []
    for c in range(N_CORES):
        xc = X[c * TOK_C:(c + 1) * TOK_C]  # [TOK_C, IN_F]
        # xt[p, j, t] = x[t, j*128+p]
        xtc = np.ascontiguousarray(
            xc.reshape(TOK_C, KT, P).transpose(2, 1, 0).astype(np.float16)
        )
        in_maps.append({"xt": xtc, "wt": wp, "bias": bias2, "scale": scale2})
    return in_maps


def assemble_output(results):
    out = np.concatenate([results[c]["out"] for c in range(N_CORES)], axis=0)
    return np.ascontiguousarray(out.reshape(B, S, OUT_F).astype(np.float32))


def kernel(input, qweight, weight_scale, bias_param):
    from concourse.bass_utils import run_bass_kernel_spmd

    in_maps = prep_inputs(input, qweight, weight_scale, bias_param)
    nc = build_nc()
    res = run_bass_kernel_spmd(nc, in_maps, core_ids=list(range(N_CORES)))
    return assemble_output(res.results)
